# revision 18
# baseline (speedup 1.0000x reference)
"""CharBiLSTM embedder on 8 TRN2 NeuronCores (Bass/Tile) — v2.

Words sorted by length, dealt round-robin to 8 cores (identical per-core
length profile), 9 tiles x 512 words per core.  Tiles are assigned to 3
concurrent LANES (balanced by total step count); each lane runs its tiles
back-to-back, one LSTM step per global slot.  Per lane-step:

  PE:  4 matmuls [128,live] (one per (gate-pair, dir)):
       bank0=[i_f|f_f] bank1=[i_b|f_b] bank2=[2g_f|o_f] bank3=[2g_b|o_b]
       (g-gates pre-scaled x2 so sigmoid(2g) = (tanh(g)+1)/2; h stored as
       h' = h/2 so W_hh is folded x2; rhs = per-dir char-embedding buffer
       with h' written into slice t+1 by the previous step.)
  Act: one SIG over all 4 banks [0:100, 4*live]; one TANH on c [0:114,live].
  DVE: t1=(s2g-.5)*si (both dirs strided); t2=sf*c x2; c=(2*t1)+t2 x2 (STT);
       h'=(th*.5)*so x2 (STT) -> written into next rhs slice.
  Pool: dma_gathers (chunked for first tiles), extraction copies (x2 scale)
       into hall at each tile's final step, final out DMA.

PSUM: rotating pool of 2 x [128, 4*512] f32 (= all 8 banks) across
lane-steps -> natural cross-lane software pipelining.  All element-wise ops
live-trimmed to columns whose word length > t (sorted => suffix).
"""
import os
import sys

os.environ.setdefault("CONCOURSE_SCRUB_NEFF_DEBUG_INFO", "1")
sys.path.insert(0, "/opt/trn_rl_repo")

from contextlib import ExitStack

import ml_dtypes
import numpy as np

import concourse.bass as bass
import concourse.mybir as mybir
import concourse.tile as tile
from concourse import bacc
from concourse.tile import add_dep_helper
from concourse.bass_utils import run_bass_kernel_spmd

N, T, E, H, V = 32768, 20, 50, 50, 200
NCORES = 8
NT = 512                  # words per tile
NTILES = 9
NWPAD = NT * NTILES       # padded words per core
BF16 = mybir.dt.bfloat16
F32 = mybir.dt.float32
I16 = mybir.dt.int16
MAXL = T + 1
W16 = MAXL * (NT // 16)   # idx cols per (tile, dir)

AF = mybir.ActivationFunctionType
SIG = AF.Sigmoid
TANH = AF.Tanh
AL = mybir.AluOpType


RING = 10                 # h/x slice ring depth per (lane, dir)
CHUNK = 5                 # gather chunk size (slices)
AHEAD = 5                 # keep gathers this many slices ahead of consumption


def build_graph(Ltl, lanes, alive, ranges):
    """Ltl: per-tile max len; lanes: tuple of tile-id sequences; alive[tl][t]:
    first live column at step t; ranges[tl]: (l, a, b) extraction runs."""
    nc = bacc.Bacc()
    wts_ext = nc.declare_dram_parameter("wts", [4, 128, 128], BF16, isOutput=False)
    tab_ext = nc.declare_dram_parameter("tab", [2, 128, 256], BF16, isOutput=False)
    gidx_ext = nc.declare_dram_parameter(
        "gidx", [128, NTILES, 2, W16], I16, isOutput=False
    )
    out_ext = nc.declare_dram_parameter("out", [100, NWPAD], F32, isOutput=True)

    NL = len(lanes)
    lane_prog = []  # lane -> list of (tile, t, base) per slot
    lane_chunks = []  # lane -> list of (tile, s0, s1, base) gather chunks
    for ln in lanes:
        prog = []
        chunks = []
        base = 0
        for tl in ln:
            for t in range(Ltl[tl]):
                prog.append((tl, t, base))
            s = 0
            first_chunk = len(chunks) == 0
            while s < Ltl[tl]:
                # split chunks at ring wrap so gather output stays contiguous
                room = RING - (base + s) % RING
                sz = 2 if first_chunk else CHUNK
                first_chunk = False
                s1 = min(s + sz, Ltl[tl], s + room)
                chunks.append((tl, s, s1, base))
                s = s1
            base += Ltl[tl] + 1  # +1: the final h' slice occupies one slot
        lane_prog.append(prog)
        lane_chunks.append(chunks)
    SLOTS = max(len(p) for p in lane_prog)
    # per-tile extraction runs keyed by length
    ext_runs = []
    for tl in range(NTILES):
        m = {}
        for (lrun, ra, rb) in ranges[tl]:
            m.setdefault(lrun, []).append((ra, rb))
        ext_runs.append(m)

    with tile.TileContext(nc) as tc, ExitStack() as ctx:
        cpool = ctx.enter_context(tc.tile_pool(name="const", bufs=1))
        pspool = ctx.enter_context(tc.tile_pool(name="ps", bufs=2, space="PSUM"))
        sigpool = [
            ctx.enter_context(tc.tile_pool(name=f"sg{i}", bufs=2)) for i in range(NL)
        ]
        cpool_l = [
            ctx.enter_context(tc.tile_pool(name=f"c{i}", bufs=2)) for i in range(NL)
        ]
        thpool = [
            ctx.enter_context(tc.tile_pool(name=f"th{i}", bufs=2)) for i in range(NL)
        ]
        t1pool = [
            ctx.enter_context(tc.tile_pool(name=f"t1{i}", bufs=2)) for i in range(NL)
        ]
        t2pool = [
            ctx.enter_context(tc.tile_pool(name=f"t2{i}", bufs=2)) for i in range(NL)
        ]

        wts_sb = cpool.tile([128, 4 * 128], BF16, tag="wts", name="wts_sb")
        nc.sync.dma_start(
            wts_sb[:].rearrange("k (i m) -> k i m", i=4),
            wts_ext[:].rearrange("i k m -> k i m"),
        )
        tab_sb = cpool.tile([128, 512], BF16, tag="tab", name="tab_sb")
        nc.sync.dma_start(
            tab_sb[:].rearrange("k (i m) -> k i m", i=2),
            tab_ext[:].rearrange("i k m -> k i m"),
        )
        tabs = (tab_sb[:, 0:256], tab_sb[:, 256:512])
        idx_sb = cpool.tile([128, NTILES * 2 * W16], I16, tag="idx", name="idx_sb")
        idx3 = idx_sb[:].rearrange("p (t d w) -> p t d w", t=NTILES, d=2)
        first_tiles = [ln[0] for ln in lanes]
        idx_order = first_tiles + [
            tl for tl in range(NTILES)
            if tl not in first_tiles and Ltl[tl] > 0
        ]
        for tl in idx_order:
            for d in range(2):
                nc.sync.dma_start(idx3[:, tl, d, :], gidx_ext[:, tl, d, :])
        hall = cpool.tile([128, NTILES * NT], F32, tag="hall", name="hall")
        for tl in range(NTILES):
            if Ltl[tl] == 0:
                nc.vector.memset(hall[0:50, tl * NT : (tl + 1) * NT], 0.0)
                nc.vector.memset(hall[64:114, tl * NT : (tl + 1) * NT], 0.0)
                nc.sync.dma_start(
                    out_ext[0:50, tl * NT : (tl + 1) * NT],
                    hall[0:50, tl * NT : (tl + 1) * NT])
                nc.sync.dma_start(
                    out_ext[50:100, tl * NT : (tl + 1) * NT],
                    hall[64:114, tl * NT : (tl + 1) * NT])

        # persistent per-(lane, dir) slice rings
        ring_f = [
            cpool.tile([128, RING * NT], BF16, tag=f"rf{l}", name=f"rf{l}")
            for l in range(NL)
        ]
        ring_b = [
            cpool.tile([128, RING * NT], BF16, tag=f"rb{l}", name=f"rb{l}")
            for l in range(NL)
        ]

        prev = {"pe": None, "act": None, "dve": None, "pool": None}

        def emit_chunk(l, ch):
            tl, s0, s1, base = ch
            r0 = (base + s0) % RING
            n = (s1 - s0) * NT
            assert r0 + (s1 - s0) <= RING
            for d, ring in ((0, ring_f[l]), (1, ring_b[l])):
                g = nc.gpsimd.dma_gather(
                    out_ap=ring[:, r0 * NT : r0 * NT + n].rearrange(
                        "p (o n) -> p o n", o=1
                    ),
                    in_ap=tabs[d],
                    idxs_ap=idx3[:, tl, d, s0 * 32 : s1 * 32],
                    num_idxs=n,
                    num_idxs_reg=n,
                    elem_size=128,
                    transpose=True,
                    sbuf_tokens_per_rank=128,
                    sbuf_free_dim_per_rank=256,
                    sbuf_free_dim_pad_per_rank=0,
                    sbuf_byte_offset=0,
                    single_packet=False,
                )
                prev["pool"] = g

        chunk_ptr = [0] * NL       # next chunk to emit per lane
        slices_done = [0] * NL     # gathered slices (in base+t units) per lane
        cbufs = [None] * NL


        # initial gathers
        for l0 in range(NL):
            while (
                chunk_ptr[l0] < len(lane_chunks[l0])
                and slices_done[l0] < AHEAD
            ):
                ch = lane_chunks[l0][chunk_ptr[l0]]
                emit_chunk(l0, ch)
                slices_done[l0] = ch[3] + ch[2]
                chunk_ptr[l0] += 1

        # flat round-robin sequence of lane-steps (one per subslot)
        seq = []
        for k in range(SLOTS):
            for l in range(NL):
                if k < len(lane_prog[l]):
                    seq.append((l,) + lane_prog[l][k])

        def rslot(base, s):
            return ((base + s) % RING) * NT

        def emit_tail(ent, sg, th):
            """TANH consumed -> h' writes + extraction for a finished front."""
            l, tl, t, base = ent
            a = alive[tl][t]
            s2 = sg[:].rearrange("p (s n) -> p s n", s=4)
            r1 = rslot(base, t + 1)
            nc.vector.tensor_mul(
                ring_f[l][0:50, r1 + a : r1 + NT],
                th[0:50, a:NT], s2[64:114, 2, a:NT],
            )
            nc.vector.tensor_mul(
                ring_b[l][64:114, r1 + a : r1 + NT],
                th[64:114, a:NT], s2[64:114, 3, a:NT],
            )
            if t == 0 and 0 in ext_runs[tl]:
                r0 = rslot(base, 0)
                for (ra, rb) in ext_runs[tl][0]:
                    nc.gpsimd.tensor_copy(
                        hall[0:50, tl * NT + ra : tl * NT + rb],
                        ring_f[l][0:50, r0 + ra : r0 + rb],
                    )
                    nc.gpsimd.tensor_copy(
                        hall[64:114, tl * NT + ra : tl * NT + rb],
                        ring_b[l][64:114, r0 + ra : r0 + rb],
                    )
            if (t + 1) in ext_runs[tl]:
                for (ra, rb) in ext_runs[tl][t + 1]:
                    nc.gpsimd.tensor_copy(
                        hall[0:50, tl * NT + ra : tl * NT + rb],
                        ring_f[l][0:50, r1 + ra : r1 + rb],
                    )
                    nc.gpsimd.tensor_copy(
                        hall[64:114, tl * NT + ra : tl * NT + rb],
                        ring_b[l][64:114, r1 + ra : r1 + rb],
                    )

        pendq = []  # (ent, sg, cbuf) awaiting TANH + tail (depth 1)

        CHAINED = {"pe", "act"}

        def chain(key, ins):
            if key in CHAINED and prev[key] is not None:
                add_dep_helper(ins.ins, prev[key].ins, sync=False,
                               reason=f"{key} order")
            prev[key] = ins
            return ins

        def emit_front_tail(i, which, ent, sg=None, cbuf=None):
            """which='tail': TANH + h' + extraction for ent."""
            l, tl, t, base = ent
            a = alive[tl][t]
            s2 = sg[:].rearrange("p (s n) -> p s n", s=4)
            th = thpool[l].tile([128, NT], BF16, tag="th", name=f"th{i}")
            chain("act", nc.scalar.activation(
                th[0:114, a:NT], cbuf[0:114, a:NT], TANH))
            r1 = rslot(base, t + 1)
            chain("dve", nc.vector.tensor_mul(
                ring_f[l][0:50, r1 + a : r1 + NT],
                th[64:114, a:NT], s2[64:114, 2, a:NT]))
            chain("dve", nc.vector.tensor_mul(
                ring_b[l][64:114, r1 + a : r1 + NT],
                th[0:50, a:NT], s2[0:50, 3, a:NT]))
            if t == 0 and 0 in ext_runs[tl]:
                r0 = rslot(base, 0)
                for (ra, rb) in ext_runs[tl][0]:
                    chain("pool", nc.gpsimd.tensor_copy(
                        hall[0:50, tl * NT + ra : tl * NT + rb],
                        ring_f[l][0:50, r0 + ra : r0 + rb]))
                    chain("pool", nc.gpsimd.tensor_copy(
                        hall[64:114, tl * NT + ra : tl * NT + rb],
                        ring_b[l][64:114, r0 + ra : r0 + rb]))
            if (t + 1) in ext_runs[tl]:
                for (ra, rb) in ext_runs[tl][t + 1]:
                    chain("pool", nc.gpsimd.tensor_copy(
                        hall[0:50, tl * NT + ra : tl * NT + rb],
                        ring_f[l][0:50, r1 + ra : r1 + rb]))
                    chain("pool", nc.gpsimd.tensor_copy(
                        hall[64:114, tl * NT + ra : tl * NT + rb],
                        ring_b[l][64:114, r1 + ra : r1 + rb]))
            if t == Ltl[tl] - 1:
                nc.sync.dma_start(
                    out_ext[0:50, tl * NT : (tl + 1) * NT],
                    hall[0:50, tl * NT : (tl + 1) * NT])
                nc.sync.dma_start(
                    out_ext[50:100, tl * NT : (tl + 1) * NT],
                    hall[64:114, tl * NT : (tl + 1) * NT])

        lane_pos = [0] * NL
        for i, ent in enumerate(seq):
            l, tl, t, base = ent
            a = alive[tl][t]
            lane_pos[l] = base + t
            if t == 0:
                cbuf = cpool_l[l].tile([128, NT], BF16, tag="c", name=f"c{l}_{tl}")
                chain("pool", nc.gpsimd.memset(cbuf[:], 0.0))
                cbufs[l] = cbuf
            cbuf = cbufs[l]

            ps = pspool.tile([128, 4 * NT], F32, tag="ps", name=f"ps{i}")
            r0 = rslot(base, t)
            for w in range(4):
                w_ap = wts_sb[:, w * 128 : (w + 1) * 128]
                ring = ring_f[l] if w % 2 == 0 else ring_b[l]
                rhs = ring[:, r0 + a : r0 + NT]
                chain("pe", nc.tensor.matmul(
                    ps[:, w * NT + a : (w + 1) * NT], w_ap, rhs,
                    start=True, stop=True))

            sg = sigpool[l].tile([128, 4 * NT], BF16, tag="sg", name=f"sg{i}")
            chain("act", nc.scalar.activation(
                sg[0:114, :].rearrange("p (s n) -> p s n", s=4)[:, :, a:NT],
                ps[0:114, :].rearrange("p (s n) -> p s n", s=4)[:, :, a:NT],
                SIG))

            # tail of the PREVIOUS subslot
            if pendq:
                pent, psg, pcbuf = pendq.pop(0)
                emit_front_tail(i, "tail", pent, psg, pcbuf)

            t1 = t1pool[l].tile([128, 2 * NT], BF16, tag="t1", name=f"t1{i}")
            t2 = t2pool[l].tile([128, 2 * NT], BF16, tag="t2", name=f"t2{i}")
            s2 = sg[:].rearrange("p (s n) -> p s n", s=4)
            t1v = t1[:].rearrange("p (s n) -> p s n", s=2)
            # banks: b0=[i_f|f_f] b1=[f_b|i_b] b2=[2g_f|o_f] b3=[2g_b|o_b];
            # c packed [c_b 0:50 | c_f 64:114]; all 2-input ops have equal
            # input base partitions (HW verifier constraint); partition moves
            # happen only via op OUTPUTS.
            # t2 = sf * c  (per dir, on Pool; needs only SIG + prev c)
            chain("pool", nc.gpsimd.tensor_mul(
                t2[64:114, a:NT], s2[64:114, 0, a:NT], cbuf[64:114, a:NT]))
            chain("pool", nc.gpsimd.tensor_mul(
                t2[0:50, NT + a : 2 * NT], s2[0:50, 1, a:NT],
                cbuf[0:50, a:NT]))
            # tg = 2*s2g - 1 = tanh(g)  (tensor_scalar, inputs at base 0)
            chain("dve", nc.vector.tensor_scalar(
                t1[0:50, a:NT], s2[0:50, 2, a:NT], 2.0, -1.0,
                AL.mult, AL.add))
            chain("dve", nc.vector.tensor_scalar(
                t1[64:114, NT + a : 2 * NT], s2[64:114, 3, a:NT], 2.0, -1.0,
                AL.mult, AL.add))
            # t1 = tg * si  (outputs move partitions; inputs aligned)
            chain("dve", nc.vector.tensor_mul(
                t1[64:114, a:NT], t1[0:50, a:NT], s2[0:50, 0, a:NT]))
            chain("dve", nc.vector.tensor_mul(
                t1[0:50, NT + a : 2 * NT], t1[64:114, NT + a : 2 * NT],
                s2[64:114, 1, a:NT]))
            # c = t1 + t2  (per dir)
            chain("dve", nc.vector.tensor_add(
                cbuf[64:114, a:NT], t1[64:114, a:NT], t2[64:114, a:NT]))
            chain("dve", nc.vector.tensor_add(
                cbuf[0:50, a:NT], t1[0:50, NT + a : 2 * NT],
                t2[0:50, NT + a : 2 * NT]))
            pendq.append((ent, sg, cbuf))

            # prefetch gathers (Pool, after this subslot's latency-critical ops)
            for ch_l in range(NL):
                while (
                    chunk_ptr[ch_l] < len(lane_chunks[ch_l])
                    and slices_done[ch_l] < lane_pos[ch_l] + AHEAD
                ):
                    ch = lane_chunks[ch_l][chunk_ptr[ch_l]]
                    emit_chunk(ch_l, ch)
                    slices_done[ch_l] = ch[3] + ch[2]
                    chunk_ptr[ch_l] += 1

        # flush remaining tails
        for j, (pent, psg, pcbuf) in enumerate(pendq):
            emit_front_tail(10000 + j, "tail", pent, psg, pcbuf)

    nc.finalize()
    _dedup_ldweights(nc)
    return nc


def _dedup_ldweights(nc):
    """Drop consecutive PE Ldweights that reload the identical stationary."""
    PE = mybir.EngineType.PE
    removed = 0
    for blk in nc.m.functions[0].blocks:
        il = blk.instructions
        cur = None
        drop = []
        for idx, inst in enumerate(il):
            if getattr(inst, "engine", None) != PE:
                continue
            if type(inst).__name__ != "InstLdweights":
                continue
            key = repr(inst.ins[0])
            si = inst.sync_info
            waits = list(si.on_wait) if si is not None else []
            upds = list(si.on_update) if si is not None else []
            if key == cur and not upds:
                if not waits:
                    drop.append(idx)
                    continue
                nxt = None
                for j in range(idx + 1, len(il)):
                    if getattr(il[j], "engine", None) == PE:
                        nxt = il[j]
                        break
                if nxt is not None:
                    nsi = nxt.sync_info
                    nwaits = list(nsi.on_wait) if nsi is not None else []
                    if len(nwaits) + len(waits) <= 1:
                        if nsi is None:
                            nxt.sync_info = mybir.SyncInfo(
                                on_wait=waits, on_update=[]
                            )
                        else:
                            nsi.on_wait = nwaits + waits
                        drop.append(idx)
                        continue
                continue
            cur = key
        for idx in reversed(drop):
            del il[idx]
        removed += len(drop)
    return removed


def _balance_lanes(Ltl, nl=3):
    """Assign active tiles to nl lanes minimizing max lane total (brute force)."""
    act = [tl for tl in range(NTILES) if Ltl[tl] > 0]
    best = None
    best_max = 1 << 30

    def rec(i, sums, assign):
        nonlocal best, best_max
        if max(sums) >= best_max:
            return
        if i == len(act):
            best = list(assign)
            best_max = max(sums)
            return
        tl = act[i]
        for l in range(nl):
            sums[l] += Ltl[tl]
            assign.append(l)
            rec(i + 1, sums, assign)
            assign.pop()
            sums[l] -= Ltl[tl]

    # order tiles by descending length for better pruning
    act.sort(key=lambda tl: -Ltl[tl])
    rec(0, [0] * nl, [])
    lanes = [[] for _ in range(nl)]
    for tl, l in zip(act, best):
        lanes[l].append(tl)
    # run longest tile first within each lane
    for ln in lanes:
        ln.sort(key=lambda tl: -Ltl[tl])
    lanes = [tuple(ln) for ln in lanes if ln]
    return tuple(sorted(lanes, key=lambda ln: -sum(Ltl[t] for t in ln)))


def prepare_host(inputs):
    ci = np.asarray(inputs["char_indices"])
    lens = np.asarray(inputs["word_lengths"]).astype(np.int64)
    emb = np.array(inputs["emb"], dtype=np.float32)
    emb[0] = 0.0

    # --- padded, sorted word list with per-core-identical length profile ---
    order = np.argsort(lens, kind="stable")
    counts = np.bincount(lens, minlength=T + 1)
    dup_ids = []
    for l in range(T + 1):
        rem = counts[l] % 8
        if rem:
            w = order[np.searchsorted(lens[order], l)]
            dup_ids += [w] * (8 - rem)
    front = NWPAD * NCORES - N - len(dup_ids)
    assert front >= 0 and front % 8 == 0
    shortest = order[0]
    all_ids = np.concatenate(
        [order, np.array(dup_ids + [shortest] * front, dtype=np.int64)]
    )
    words_pad = all_ids[np.argsort(lens[all_ids], kind="stable")]
    plens = lens[words_pad]
    assert (plens.reshape(-1, 8).max(1) == plens.reshape(-1, 8).min(1)).all()
    prof = plens[::8].astype(np.int64)          # per-core length profile [NWPAD]

    Ltl = tuple(int(prof[(tl + 1) * NT - 1]) for tl in range(NTILES))
    lanes = _balance_lanes(Ltl)
    ranges = []
    alive = []
    for tl in range(NTILES):
        seg = prof[tl * NT : (tl + 1) * NT]
        runs = []
        a = 0
        for p in range(1, NT + 1):
            if p == NT or seg[p] != seg[a]:
                runs.append((int(seg[a]), a, p))
                a = p
        ranges.append(tuple(runs))
        # first live column at step t = first word with len > t, 4-aligned
        al = tuple(
            int(np.searchsorted(seg, t + 1, side="left")) & ~3
            for t in range(max(Ltl[tl], 1))
        )
        alive.append(al)
    ranges = tuple(ranges)
    alive = tuple(alive)

    # --- weights: 4 stationaries [128,128]: [Af, Ab, Bf, Bb] -------------
    # pair A = gates (i, f); pair B = gates (g, o); g-rows scaled x2;
    # W_hh scaled x2 globally (h stored as h/2).
    # torch gate-row order: i 0:50, f 50:100, g 100:150, o 150:200.
    gate_rows = {"i": slice(0, 50), "f": slice(50, 100),
                 "g": slice(100, 150), "o": slice(150, 200)}
    pairs = (("i", "f"), ("g", "o"))
    wts = np.zeros((4, 128, 128), np.float32)
    for pi, (g1, g2) in enumerate(pairs):
        for half, sfx in enumerate("fb"):
            Wih = np.asarray(inputs[f"W_ih_{sfx}"], dtype=np.float32).copy()
            Whh = np.asarray(inputs[f"W_hh_{sfx}"], dtype=np.float32).copy()
            bias = (
                np.asarray(inputs[f"b_ih_{sfx}"], dtype=np.float32)
                + np.asarray(inputs[f"b_hh_{sfx}"], dtype=np.float32)
            ).copy()
            Wih[gate_rows["g"]] *= 2.0           # sigmoid(2g) trick
            Whh[gate_rows["g"]] *= 2.0
            bias = bias.copy()
            bias[gate_rows["g"]] *= 2.0
            w = wts[2 * pi + half]
            swap = half == 1   # b1 = [f_b | i_b]; b3 = [o_b | 2g_b]
            for ci_, gname in enumerate((g1, g2)):
                r = gate_rows[gname]
                first = (ci_ == 0) != swap
                mc = slice(0, 50) if first else slice(64, 114)
                if half == 0:   # f-dir rhs: h' at K 0:50, x at 64:114, 1 at 127
                    w[0:50, mc] = Whh[r].T
                    w[64:114, mc] = Wih[r].T
                    w[127, mc] = bias[r]
                else:           # b-dir rhs: x at K 0:50, 1 at 50, h' at 64:114
                    w[0:50, mc] = Wih[r].T
                    w[50, mc] = bias[r]
                    w[64:114, mc] = Whh[r].T
    wts_bf = wts.astype(ml_dtypes.bfloat16)

    tab = np.zeros((2, 128, 256), np.float32)
    for v in range(V):
        rank, tok = v // 128, v % 128
        tab[0, tok, rank * 128 + 64 : rank * 128 + 114] = emb[v]
        tab[0, tok, rank * 128 + 127] = 1.0
        tab[1, tok, rank * 128 + 0 : rank * 128 + 50] = emb[v]
        tab[1, tok, rank * 128 + 50] = 1.0
    tab_bf = tab.astype(ml_dtypes.bfloat16)

    def wrap128(flat):
        # [L*NT] -> [128, L*NT//16]: wrapped in 16 partitions, replicated x8
        a = flat.reshape(-1, 16).T.astype(np.int16)
        return np.tile(a, (8, 1))

    in_maps = []
    cores_meta = []
    for c in range(NCORES):
        widx = words_pad[c::NCORES]
        ci_c = ci[widx]
        len_c = lens[widx]
        gidx = np.zeros((NTILES, 2, 128, W16), np.int16)
        for tl in range(NTILES):
            Lg = Ltl[tl]
            if Lg == 0:
                continue
            cw = ci_c[tl * NT : (tl + 1) * NT]          # [NT, T]
            lw = len_c[tl * NT : (tl + 1) * NT]          # [NT]
            tt = np.arange(Lg)
            f_chars = cw[:, :Lg].T                       # [Lg, NT]
            b_pos = np.maximum(lw[None, :] - 1 - tt[:, None], 0)
            b_chars = cw[np.arange(NT)[None, :], b_pos]  # [Lg, NT]
            gidx[tl, 0, :, : Lg * (NT // 16)] = wrap128(f_chars.reshape(-1))
            gidx[tl, 1, :, : Lg * (NT // 16)] = wrap128(b_chars.reshape(-1))
        in_maps.append(
            {"wts": wts_bf, "tab": tab_bf,
             "gidx": np.ascontiguousarray(np.transpose(gidx, (2, 0, 1, 3)))}
        )
        cores_meta.append(widx)
    return Ltl, lanes, alive, ranges, in_maps, cores_meta


_GRAPH_CACHE = {}
TRACE = False
LAST_RESULT = None


def kernel(**inputs):
    Ltl, lanes, alive, ranges, in_maps, cores_meta = prepare_host(inputs)
    key = (Ltl, lanes, alive, ranges)
    if key not in _GRAPH_CACHE:
        _GRAPH_CACHE[key] = build_graph(Ltl, lanes, alive, ranges)
    nc = _GRAPH_CACHE[key]
    global LAST_RESULT
    res = run_bass_kernel_spmd(
        nc, in_maps, core_ids=list(range(NCORES)), trace=TRACE
    )
    LAST_RESULT = res
    out = np.zeros((N, 2 * H), np.float32)
    for c in range(NCORES):
        out[cores_meta[c]] = res.results[c]["out"].T
    return out
